# revision 3
# baseline (speedup 1.0000x reference)
"""Trainium2 Bass kernel for Llama-style GQA attention (nn_LlamaAttention), v2.

Shapes (hardcoded from the problem spec):
  hidden_states [2, 2048, 4096] f32, attention_mask [2, 1, 2048, 2048] f32,
  position_ids [2, 2048] i64, Wq [4096, 4096], Wk/Wv [4096, 1024], Wo [4096, 4096].

Sharding: tensor-parallel over heads across 8 NeuronCores. Core c owns
Q heads 4c..4c+3 and KV head c (GQA groups align). Each core computes a
full-shape partial output (attn_out_c @ Wo_c) in fp16; the host sums the 8
partials in f32 (the TP all-reduce) and reshapes.

v2 vs v1 (all fp16 instead of bf16 — same PE rate, 8x the mantissa):
  - All weights host-repacked to partition-major [128, ...] layouts and
    preloaded into SBUF once per rep (4+4 big DMAs) instead of re-streamed
    from DRAM every token chunk; X^T likewise repacked so phase B issues one
    128KB DMA per k-tile.  (v1 was HWDGE descriptor-rate bound: 4 DMAs per
    1.3us of matmul.)
  - RoPE trig tables shared between q and k (the 1/sqrt(D) score scale moved
    into the exp activation's scale operand), stored per-batch [128, 2048]
    fp16; RoPE itself is 4 DVE ops per head via scalar_tensor_tensor folding
    the rotate-half negation.
  - Softmax normalization uses a single fp16 reciprocal broadcast (one K=1
    ones-matmul) instead of the hi+lo pair.
  - Phase D stages a whole token-row [128, 4096] fp16 and writes it as one
    DMA (32 output DMAs instead of 256 x 256KB f32).
  - attn-out tiles are resident so phase C/D overlap; V psum is evicted
    before the Q/K psums so the PE-transpose of V never stalls the PE FIFO.
"""
import sys
sys.path.insert(0, "/opt/trn_rl_repo")
import numpy as np

import concourse.bass as bass
import concourse.bacc as bacc
import concourse.mybir as mybir
import concourse.tile as tile

F32 = mybir.dt.float32
FP16 = mybir.dt.float16
AF = mybir.ActivationFunctionType
ALU = mybir.AluOpType

H = 4096
NH = 32
NKV = 8
D = 128
B = 2
S = 2048
T = B * S
NC = 8
HQ = NH // NC          # 4 q heads per core
QCOLS = HQ * D         # 512
ROPE_BASE = 10000.0
NQC = S // 512         # 4 q-chunks of 512 per batch
NKT = S // 128         # 16 k-tiles of 128 per batch
NKG = H // 128         # 32 contraction k-tiles
NCH = T // 512         # 8 token chunks
MASK_PRELOAD_MAX = 24
SM = float(1.0 / np.sqrt(D))


def _build_program(plan, n_uniq, nreps, phases=(1, 1, 1)):
    """plan[b][qc] = tuple of (kt, mask_idx) with mask_idx == -1 for free blocks."""
    nc = bacc.Bacc(None, target_bir_lowering=False)

    # X^T repacked [128, (n*32+k)*512+j] = X[n*512+j, k*128+p]
    xtr_d = nc.dram_tensor("xtr", [128, NCH * NKG * 512], FP16, kind="ExternalInput")
    # weights repacked partition-major, k-tile-major
    wq_d = nc.dram_tensor("wq", [128, NKG * QCOLS], FP16, kind="ExternalInput")
    wk_d = nc.dram_tensor("wk", [128, NKG * D], FP16, kind="ExternalInput")
    wv_d = nc.dram_tensor("wv", [128, NKG * D], FP16, kind="ExternalInput")
    wo_d = nc.dram_tensor("wo", [128, HQ * H], FP16, kind="ExternalInput")
    cos_d = nc.dram_tensor("cosb", [D, S], FP16, kind="ExternalInput")
    sin_d = nc.dram_tensor("sinb", [D, S], FP16, kind="ExternalInput")
    nmask = max(n_uniq, 1)
    masks_d = nc.dram_tensor("masks", [nmask, 128, 1024], FP16, kind="ExternalInput")
    onescol_d = nc.dram_tensor("onescol", [128, 1], FP16, kind="ExternalInput")
    onesrow_d = nc.dram_tensor("onesrow", [1, 128], FP16, kind="ExternalInput")
    ident_d = nc.dram_tensor("ident", [128, 128], FP16, kind="ExternalInput")
    out_d = nc.dram_tensor("out", [T, H], FP16, kind="ExternalOutput")

    preload_masks = n_uniq > 0 and n_uniq <= MASK_PRELOAD_MAX
    NWG = NKG // 4  # 8 k-tiles per weight group

    # queue-mode SBUF pool allocation: successive phases' pools get distinct
    # addresses, so phase C's tiles don't carry a WAR dependency on phase B's
    # still-draining staging/rope tiles (and likewise C -> D).
    with tile.TileContext(nc, pool_alloc_mode="queue") as tc:

        def phase_b(wqs, wks, wvs, cosb, sinb, qt, ktr, vsb, idt, late_preloads):
            with tc.tile_pool(name="xtp", bufs=6) as xtp, \
                 tc.tile_pool(name="pbps", bufs=1, space="PSUM") as pbps, \
                 tc.tile_pool(name="tps", bufs=2, space="PSUM") as tps, \
                 tc.tile_pool(name="stg", bufs=2) as stg, \
                 tc.tile_pool(name="rope", bufs=3) as ropep:
                for n in range(NCH):
                    tok = slice(n * 512, (n + 1) * 512)
                    poff = (n % NQC) * 512
                    ps_q = [pbps.tile([128, 512], F32, tag=f"pq{m}", name=f"psq{m}")
                            for m in range(HQ)]
                    ps_k = pbps.tile([128, 512], F32, tag="pk", name="psk")
                    ps_v = pbps.tile([128, 512], F32, tag="pv", name="psv")
                    for k in range(NKG):
                        # issue the remaining resident-tile preloads at the
                        # program points just before their data is needed so
                        # the first matmuls aren't queued behind 10MB of DMA
                        for fn in late_preloads.pop((n, k), ()):
                            fn()
                        g, kk = k // NWG, k % NWG
                        xt_t = xtp.tile([128, 512], FP16, tag="xt", name="xt_t")
                        nc.sync.dma_start(
                            xt_t[:], xtr_d[:, (n * NKG + k) * 512:(n * NKG + k + 1) * 512])
                        st = (k == 0)
                        sp = (k == NKG - 1)
                        nc.tensor.matmul(ps_v[:], wvs[g][:, kk * D:(kk + 1) * D],
                                         xt_t[:], start=st, stop=sp)
                        nc.tensor.matmul(ps_k[:], wks[g][:, kk * D:(kk + 1) * D],
                                         xt_t[:], start=st, stop=sp)
                        for m in range(HQ):
                            nc.tensor.matmul(
                                ps_q[m][:],
                                wqs[g][:, kk * QCOLS + m * 128:kk * QCOLS + (m + 1) * 128],
                                xt_t[:], start=st, stop=sp)

                    # evict psums to fp16 staging via ScalarE; V first so the
                    # PE transposes (next in the PE FIFO) are not stalled.  On
                    # the final chunk evict the q psums first instead: phase C's
                    # first score tiles reuse those banks.
                    stv = stg.tile([128, 512], FP16, tag="sv", name="stv")
                    stk = stg.tile([128, 512], FP16, tag="sk", name="stk")
                    stq = [stg.tile([128, 512], FP16, tag=f"sq{m}", name=f"stq{m}")
                           for m in range(HQ)]
                    if n == NCH - 1:
                        nc.scalar.copy(stq[0][:], ps_q[0][:])
                        nc.scalar.copy(stq[1][:], ps_q[1][:])
                        nc.scalar.copy(stv[:], ps_v[:])
                        nc.scalar.copy(stq[2][:], ps_q[2][:])
                        nc.scalar.copy(stq[3][:], ps_q[3][:])
                        nc.scalar.copy(stk[:], ps_k[:])
                    else:
                        nc.scalar.copy(stv[:], ps_v[:])
                        nc.scalar.copy(stk[:], ps_k[:])
                        for m in range(HQ):
                            nc.scalar.copy(stq[m][:], ps_q[m][:])

                    # V: PE-transpose fp16 staging to token-major
                    for j in range(4):
                        ktg = 4 * n + j
                        tp_t = tps.tile([128, 128], FP16, tag="tp", name="tp_t")
                        nc.tensor.transpose(tp_t[:], stv[:, j * 128:(j + 1) * 128], idt[:])
                        nc.scalar.copy(vsb[:, ktg * 128:(ktg + 1) * 128], tp_t[:])

                    # RoPE on DVE: dst = x*cos + rotate_half(x)*sin, all fp16.
                    cs = cosb[:, poff:poff + 512]
                    sn = sinb[:, poff:poff + 512]

                    def rope(dst, x, nm):
                        rot = ropep.tile([128, 512], FP16, tag="rot", name=f"rot{nm}")
                        nc.vector.tensor_scalar_mul(rot[0:64, :], x[64:128, :], -1.0)
                        nc.vector.tensor_copy(rot[64:128, :], x[0:64, :])
                        t2 = ropep.tile([128, 512], FP16, tag="t2", name=f"t2{nm}")
                        nc.vector.tensor_tensor(t2[:], rot[:], sn, ALU.mult)
                        t1 = ropep.tile([128, 512], FP16, tag="t1", name=f"t1{nm}")
                        nc.vector.tensor_tensor(t1[:], x[:], cs, ALU.mult)
                        nc.vector.tensor_tensor(dst, t1[:], t2[:], ALU.add)

                    rope(ktr[:, tok], stk[:], "k")
                    for m in range(HQ):
                        rope(qt[m][:, tok], stq[m][:], f"q{m}")

        def phase_c(qt, ktr, vsb, onc, onr, aot, mres, masks_stream):
            # scps declared first so its 4KB tiles are 2-bank aligned (each
            # matmul output must stay inside one 2KB bank).
            with tc.tile_pool(name="scps", bufs=3, space="PSUM") as scps, \
                 tc.tile_pool(name="aops", bufs=1, space="PSUM") as aops, \
                 tc.tile_pool(name="dnps", bufs=1, space="PSUM") as dnps, \
                 tc.tile_pool(name="atp", bufs=4) as atp, \
                 tc.tile_pool(name="etp", bufs=2) as etp, \
                 tc.tile_pool(name="accp", bufs=2) as accp, \
                 tc.tile_pool(name="aosp", bufs=2) as aosp, \
                 tc.tile_pool(name="bcp", bufs=2) as bcp, \
                 tc.tile_pool(name="mskp", bufs=3) as mskp, \
                 tc.tile_pool(name="rcp", bufs=2) as rcp:
                for b in range(B):
                    for h in range(HQ):
                        # qc descending: qc=3 leads with 12 unmasked pairs, so
                        # phase C's first AV matmuls don't queue behind the
                        # final RoPE chunk still draining on the DVE
                        for qc in reversed(range(NQC)):
                            qs = slice(b * S + qc * 512, b * S + (qc + 1) * 512)
                            pairs = plan[b][qc]
                            if len(pairs) == 0:
                                nc.vector.memset(aot[h][:, qs], 0.0)
                                continue
                            last = len(pairs) - 1
                            ao_t = aops.tile([128, 512], F32, tag="ao", name="ao_t")
                            acc = accp.tile([128, 1024], FP16, tag="acc", name="acc")
                            for i, (kts, pm) in enumerate(pairs):
                                # two k-tile scores into one 2-bank psum tile,
                                # one wide exp, wide fp16 denominator accum
                                sc_t = scps.tile([128, 1024], F32, tag="sc", name="sc_t")
                                for u, kt in enumerate(kts):
                                    ksl = slice(b * S + kt * 128, b * S + kt * 128 + 128)
                                    nc.tensor.matmul(sc_t[:, u * 512:(u + 1) * 512],
                                                     ktr[:, ksl], qt[h][:, qs],
                                                     start=True, stop=True)
                                at = atp.tile([128, 1024], FP16, tag="at", name="at")
                                w = len(kts) * 512
                                if pm < 0:
                                    nc.scalar.activation(at[:, 0:w], sc_t[:, 0:w],
                                                         AF.Exp, scale=SM)
                                else:
                                    tmp = etp.tile([128, 1024], FP16, tag="etmp",
                                                   name="etmp")
                                    nc.scalar.activation(tmp[:, 0:w], sc_t[:, 0:w],
                                                         AF.Exp, scale=SM)
                                    if mres is not None:
                                        mt = mres[:, pm * 1024:pm * 1024 + w]
                                    else:
                                        mt_t = mskp.tile([128, 1024], FP16,
                                                         tag="mst", name="mst")
                                        nc.sync.dma_start(mt_t[:], masks_stream[pm])
                                        mt = mt_t[:, 0:w]
                                    nc.vector.tensor_tensor(at[:, 0:w], tmp[:, 0:w],
                                                            mt, ALU.mult)
                                if len(kts) == 1:
                                    nc.vector.memset(at[:, 512:1024], 0.0)
                                for u, kt in enumerate(kts):
                                    kg = b * NKT + kt
                                    nc.tensor.matmul(
                                        ao_t[:], vsb[:, kg * 128:(kg + 1) * 128],
                                        at[:, u * 512:(u + 1) * 512],
                                        start=(i == 0 and u == 0),
                                        stop=(i == last and u == len(kts) - 1))
                                if i == 0:
                                    nc.vector.tensor_copy(acc[:], at[:])
                                else:
                                    nc.vector.tensor_tensor(acc[:], acc[:], at[:],
                                                            ALU.add)
                            # evict the attention-output psum to SBUF right
                            # away so the single ao bank frees for the next qc;
                            # alternate DVE/ACT so neither becomes the drum.
                            ao_sb = aosp.tile([128, 512], FP16, tag="aosb", name="ao_sb")
                            if qc % 2 == 0:
                                nc.vector.tensor_copy(ao_sb[:], ao_t[:])
                            else:
                                nc.scalar.copy(ao_sb[:], ao_t[:])
                            # partition-sum of both acc halves, 1/denom, then
                            # partition-broadcast on GpSimd (no PE/ACT cost)
                            dn_t = dnps.tile([1, 512], F32, tag="dn", name="dn_t")
                            nc.tensor.matmul(dn_t[:], onc[:], acc[:, 0:512],
                                             start=True, stop=False)
                            nc.tensor.matmul(dn_t[:], onc[:], acc[:, 512:1024],
                                             start=False, stop=True)
                            rch = rcp.tile([1, 512], FP16, tag="rch", name="rch")
                            with nc.allow_low_precision(
                                    reason="fp16 1/denom: 0.05% rel err, "
                                           "well inside the 2e-2 budget"):
                                nc.vector.reciprocal(rch[:], dn_t[:])
                            bc_sb = bcp.tile([128, 512], FP16, tag="bcsb", name="bc_sb")
                            nc.gpsimd.partition_broadcast(bc_sb[:], rch[:])
                            nc.vector.tensor_tensor(aot[h][:, qs], ao_sb[:], bc_sb[:],
                                                    ALU.mult)

        def phase_d(wos, aot):
            with tc.tile_pool(name="pops", bufs=2, space="PSUM") as pops, \
                 tc.tile_pool(name="ostp", bufs=2) as ostp:
                for t in range(T // 128):
                    trows = slice(t * 128, (t + 1) * 128)
                    ost = ostp.tile([128, H], FP16, tag="ost", name="ost")
                    for half in range(2):
                        pos = [pops.tile([128, 512], F32, tag=f"po{n}", name=f"po{n}")
                               for n in range(4)]
                        for j in range(HQ):
                            for n in range(4):
                                nn_ = half * 4 + n
                                nc.tensor.matmul(
                                    pos[n][:], aot[j][:, trows],
                                    wos[j][:, nn_ * 512:(nn_ + 1) * 512],
                                    start=(j == 0), stop=(j == HQ - 1))
                        for n in range(4):
                            nn_ = half * 4 + n
                            nc.scalar.copy(ost[:, nn_ * 512:(nn_ + 1) * 512], pos[n][:])
                        if t >= T // 128 - 2:
                            # drain the tail faster: per-half DMAs on the last
                            # two token rows
                            nc.sync.dma_start(
                                out_d[trows, half * 2048:(half + 1) * 2048],
                                ost[:, half * 2048:(half + 1) * 2048])
                    if t < T // 128 - 2:
                        nc.sync.dma_start(out_d[trows, :], ost[:])

        def body(iv):
            with tc.tile_pool(name="resident", bufs=1) as rp:
                # weights, trig, constants — preloaded once per rep.  Only the
                # first contraction group's weights are loaded up front; the
                # rest are issued at need-points inside phase B so phase B's
                # first matmuls aren't queued behind 10MB of preload DMA.
                wqs = [rp.tile([128, NWG * QCOLS], FP16, name=f"wqs{g}", tag=f"wqs{g}")
                       for g in range(4)]
                wks = [rp.tile([128, NWG * D], FP16, name=f"wks{g}", tag=f"wks{g}")
                       for g in range(4)]
                wvs = [rp.tile([128, NWG * D], FP16, name=f"wvs{g}", tag=f"wvs{g}")
                       for g in range(4)]
                wos = [rp.tile([128, H], FP16, name=f"wos{j}", tag=f"wos{j}")
                       for j in range(HQ)]
                cosb = rp.tile([D, S], FP16, name="cosb")
                sinb = rp.tile([D, S], FP16, name="sinb")
                onc = rp.tile([128, 1], FP16, name="onc")
                onr = rp.tile([1, 128], FP16, name="onr")
                idt = rp.tile([128, 128], FP16, name="idt")
                mres = rp.tile([128, n_uniq * 1024], FP16, name="mres") \
                    if preload_masks else None

                def load_wg(g):
                    def fn():
                        nc.sync.dma_start(
                            wqs[g][:], wq_d[:, g * NWG * QCOLS:(g + 1) * NWG * QCOLS])
                        nc.sync.dma_start(
                            wks[g][:], wk_d[:, g * NWG * D:(g + 1) * NWG * D])
                        nc.sync.dma_start(
                            wvs[g][:], wv_d[:, g * NWG * D:(g + 1) * NWG * D])
                    return fn

                def load_trig():
                    nc.sync.dma_start(cosb[:], cos_d[:])
                    nc.sync.dma_start(sinb[:], sin_d[:])

                def load_consts():
                    nc.sync.dma_start(onc[:], onescol_d[:])
                    nc.sync.dma_start(onr[:], onesrow_d[:])
                    if mres is not None:
                        for u in range(n_uniq):
                            nc.sync.dma_start(mres[:, u * 1024:(u + 1) * 1024],
                                              masks_d[u])

                def load_wo(j):
                    def fn():
                        nc.sync.dma_start(wos[j][:], wo_d[:, j * H:(j + 1) * H])
                    return fn

                # g0 weights + the transpose identity load first (phase B needs
                # them immediately); everything else staged into phase B.
                load_wg(0)()
                nc.sync.dma_start(idt[:], ident_d[:])
                late = {
                    (0, 4): (load_wg(1),),
                    (0, 12): (load_wg(2),),
                    (0, 20): (load_wg(3), load_trig),
                    (1, 0): (load_consts,),
                    (2, 0): (load_wo(0), load_wo(1)),
                    (3, 0): (load_wo(2), load_wo(3)),
                }

                # resident activations
                qt = [rp.tile([128, T], FP16, name=f"qt{m}", tag=f"qt{m}")
                      for m in range(HQ)]
                ktr = rp.tile([128, T], FP16, name="ktr")
                vsb = rp.tile([128, T], FP16, name="vsb")
                aot = [rp.tile([128, T], FP16, name=f"aot{m}", tag=f"aot{m}")
                       for m in range(HQ)]

                if phases[0]:
                    phase_b(wqs, wks, wvs, cosb, sinb, qt, ktr, vsb, idt, late)
                else:
                    for key in sorted(late):
                        for fn in late[key]:
                            fn()
                if phases[1]:
                    phase_c(qt, ktr, vsb, onc, onr, aot,
                            mres[:] if mres is not None else None, masks_d)
                if phases[2]:
                    phase_d(wos, aot)

        if nreps == 1:
            body(0)
        else:
            with tc.For_i(0, nreps) as iv:
                body(iv)
    nc.compile()
    return nc


# ---------------------------------------------------------------------------
# Host-side preparation


def _rope_cos_sin(position_ids):
    """cos/sin in [D, S] fp16 (transposed), shared q/k, single batch.

    position_ids is [B, S] arange per batch (identical rows); falls back to
    row 0 if batches differ (they don't for this problem)."""
    inv_freq = 1.0 / (ROPE_BASE ** (np.arange(0, D, 2, dtype=np.float32) / D))
    pos = np.asarray(position_ids)[0].astype(np.float32)   # [S]
    freqs = pos[:, None] * inv_freq[None, :]               # [S, D/2]
    emb = np.concatenate([freqs, freqs], axis=-1)          # [S, D]
    cos = np.cos(emb).astype(np.float16).T                 # [D, S]
    sin = np.sin(emb).astype(np.float16).T
    return np.ascontiguousarray(cos), np.ascontiguousarray(sin)


def _classify_mask(attention_mask):
    """Pair plan over exp(mask^T) blocks [128 k, 512 q].

    Returns (plan, pair_tiles): plan[b][qc] = tuple of ((kt,...), pmask_idx)
    where each entry covers one or two k-tiles (two score tiles share one
    2-bank psum + one wide exp), pmask_idx == -1 when every covered entry is
    unmasked, else an index into pair_tiles ([128, 1024] fp16, the second
    half all-ones for a singleton pair)."""
    expm = np.exp(attention_mask[:, 0].astype(np.float32))  # [B, S, S] in [0, inf)
    blk_uniq = {}
    blk_tiles = []
    plan_blocks = []
    for b in range(B):
        planb = []
        for qc in range(NQC):
            blocks = []
            qsl = slice(qc * 512, (qc + 1) * 512)
            for kt in range(NKT):
                blk = expm[b, qsl, kt * 128:(kt + 1) * 128].T  # [128 k, 512 q]
                if not blk.any():
                    continue  # fully masked -> skip
                if (blk == 1.0).all():
                    blocks.append((kt, -1))
                    continue
                key = blk.tobytes()
                u = blk_uniq.get(key)
                if u is None:
                    u = len(blk_tiles)
                    blk_uniq[key] = u
                    blk_tiles.append(np.ascontiguousarray(blk))
                blocks.append((kt, u))
            planb.append(tuple(blocks))
        plan_blocks.append(tuple(planb))

    pair_uniq = {}
    pair_tiles = []

    def pair_mask_id(mu0, mu1):
        if mu0 < 0 and mu1 < 0:
            return -1
        key = (mu0, mu1)
        p = pair_uniq.get(key)
        if p is None:
            t = np.ones((128, 1024), np.float32)
            if mu0 >= 0:
                t[:, 0:512] = blk_tiles[mu0]
            if mu1 >= 0:
                t[:, 512:1024] = blk_tiles[mu1]
            p = len(pair_tiles)
            pair_uniq[key] = p
            pair_tiles.append(t)
        return p

    plan = []
    for b in range(B):
        planb = []
        for qc in range(NQC):
            blocks = plan_blocks[b][qc]
            pairs = []
            for i in range(0, len(blocks) - 1, 2):
                (kt0, mu0), (kt1, mu1) = blocks[i], blocks[i + 1]
                pairs.append(((kt0, kt1), pair_mask_id(mu0, mu1)))
            if len(blocks) % 2:
                kt, mu = blocks[-1]
                pairs.append(((kt,), pair_mask_id(mu, -1)))
            planb.append(tuple(pairs))
        plan.append(tuple(planb))
    return tuple(plan), pair_tiles


def _prepare_in_maps(hidden_states, attention_mask, position_ids, Wq, Wk, Wv, Wo):
    f16 = np.float16
    X = np.asarray(hidden_states, dtype=np.float32).reshape(T, H)
    # xtr[p, (n*32+k)*512+j] = X[n*512+j, k*128+p]
    xtr = np.ascontiguousarray(
        X.reshape(NCH, 512, NKG, 128).transpose(3, 0, 2, 1).reshape(128, NCH * NKG * 512)
    ).astype(f16)
    cosb, sinb = _rope_cos_sin(position_ids)
    plan, tiles = _classify_mask(np.asarray(attention_mask))
    n_uniq = len(tiles)
    masks = (np.stack(tiles) if n_uniq
             else np.zeros((1, 128, 1024), np.float32)).astype(f16)
    onescol = np.ones((128, 1), f16)
    onesrow = np.ones((1, 128), f16)
    ident = np.eye(128, dtype=f16)
    Wq = np.asarray(Wq, dtype=np.float32)
    Wk = np.asarray(Wk, dtype=np.float32)
    Wv = np.asarray(Wv, dtype=np.float32)
    Wo = np.asarray(Wo, dtype=np.float32)
    in_maps = []
    for c in range(NC):
        wq_c = Wq[:, c * QCOLS:(c + 1) * QCOLS]            # [4096, 512]
        wk_c = Wk[:, c * D:(c + 1) * D]                    # [4096, 128]
        wv_c = Wv[:, c * D:(c + 1) * D]
        wo_c = Wo[c * QCOLS:(c + 1) * QCOLS, :]            # [512, 4096]
        in_maps.append({
            "xtr": xtr,
            "wq": np.ascontiguousarray(
                wq_c.reshape(NKG, 128, QCOLS).transpose(1, 0, 2).reshape(128, -1)
            ).astype(f16),
            "wk": np.ascontiguousarray(
                wk_c.reshape(NKG, 128, D).transpose(1, 0, 2).reshape(128, -1)
            ).astype(f16),
            "wv": np.ascontiguousarray(
                wv_c.reshape(NKG, 128, D).transpose(1, 0, 2).reshape(128, -1)
            ).astype(f16),
            "wo": np.ascontiguousarray(
                wo_c.reshape(HQ, 128, H).transpose(1, 0, 2).reshape(128, -1)
            ).astype(f16),
            "cosb": cosb, "sinb": sinb,
            "masks": masks,
            "onescol": onescol, "onesrow": onesrow, "ident": ident,
        })
    return in_maps, plan, n_uniq


# ---------------------------------------------------------------------------
# Execution (modeled on concourse.bass2jax.run_bass_via_pjrt, jit kept for reuse)

_RUNNER_CACHE = {}


class SpmdRunner:
    def __init__(self, nc, n_cores=NC):
        import jax
        from jax.sharding import Mesh, PartitionSpec
        from jax.experimental.shard_map import shard_map
        from concourse.bass2jax import (_bass_exec_p, install_neuronx_cc_hook,
                                        partition_id_tensor)
        self.jax = jax
        install_neuronx_cc_hook()
        self.n_cores = n_cores
        partition_name = nc.partition_id_tensor.name if nc.partition_id_tensor else None
        in_names, out_names, out_avals = [], [], []
        for alloc in nc.m.functions[0].allocations:
            if not isinstance(alloc, mybir.MemoryLocationSet):
                continue
            name = alloc.memorylocations[0].name
            if alloc.kind == "ExternalInput":
                in_names.append(name)
            elif alloc.kind == "ExternalOutput":
                out_names.append(name)
                out_avals.append(jax.core.ShapedArray(
                    tuple(alloc.tensor_shape), mybir.dt.np(alloc.dtype)))
        if partition_name is not None and partition_name in in_names:
            in_names.remove(partition_name)
        self.in_names, self.out_names, self.out_avals = in_names, out_names, out_avals
        n_params, n_outs = len(in_names), len(out_avals)
        all_in_names = tuple(in_names + out_names)
        if partition_name is not None:
            all_in_names = all_in_names + (partition_name,)

        def _body(*args):
            operands = list(args)
            if partition_name is not None:
                operands.append(partition_id_tensor())
            outs = _bass_exec_p.bind(
                *operands, out_avals=tuple(out_avals), in_names=all_in_names,
                out_names=tuple(out_names), lowering_input_output_aliases=(),
                sim_require_finite=True, sim_require_nnan=True, nc=nc)
            return tuple(outs)

        devices = jax.devices()[:n_cores]
        mesh = Mesh(np.asarray(devices), ("core",))
        in_specs = (PartitionSpec("core"),) * (n_params + n_outs)
        out_specs = (PartitionSpec("core"),) * n_outs
        self.fn = jax.jit(
            shard_map(_body, mesh=mesh, in_specs=in_specs,
                      out_specs=out_specs, check_rep=False),
            keep_unused=True)

    def prepare(self, in_maps):
        concat_in = [
            np.concatenate([np.asarray(in_maps[c][name]) for c in range(self.n_cores)],
                           axis=0)
            for name in self.in_names
        ]
        concat_zeros = [
            np.zeros((self.n_cores * a.shape[0], *a.shape[1:]), a.dtype)
            for a in self.out_avals
        ]
        return [self.jax.device_put(a) for a in concat_in + concat_zeros]

    def run(self, args):
        outs = self.fn(*args)
        self.jax.block_until_ready(outs)
        return outs

    def results(self, outs):
        return [
            {name: np.asarray(outs[i]).reshape(self.n_cores, *self.out_avals[i].shape)[c]
             for i, name in enumerate(self.out_names)}
            for c in range(self.n_cores)
        ]


def get_runner(plan, n_uniq, nreps=1):
    key = (plan, n_uniq, nreps)
    r = _RUNNER_CACHE.get(key)
    if r is None:
        nc = _build_program(plan, n_uniq, nreps)
        r = SpmdRunner(nc)
        _RUNNER_CACHE[key] = r
    return r


def kernel(hidden_states, attention_mask, position_ids, Wq, Wk, Wv, Wo):
    in_maps, plan, n_uniq = _prepare_in_maps(
        hidden_states, attention_mask, position_ids, Wq, Wk, Wv, Wo)
    r = get_runner(plan, n_uniq, nreps=1)
    outs = r.run(r.prepare(in_maps))
    res = r.results(outs)
    full = res[0]["out"].astype(np.float32).copy()
    for c in range(1, NC):
        full += res[c]["out"]
    return full.reshape(B, S, H)
